# revision 7
# baseline (speedup 1.0000x reference)
"""CharRNN Trainium2 kernel.

Math (matches reference):
    EW = embedding @ W_e                       # [33, 200]  (device-computed)
    h_t = tanh(EW[x_t] + h_{t-1} @ W_h)        # 1024 sequential steps
    logits = h @ W_o
Sharding: data-parallel over batch, 32 rows per core across 8 cores.

On-device layout (per core): state kept transposed, g = h^T, split into
g0 = h^T[0:128] and g1 = h^T[128:200].  The embedding term is folded into
the recurrence matmul as an extra contraction chunk: the K1 weight tile is
vstack(W_h[128:200], EW) [105, 200] and its rhs is [g1; onehot(x_t)]
[105, B].  One-hots live at partitions 72:105 of the same buffer whose
rows 0:72 hold g1, so a single strided buffer serves both and the tanh
writes (rows 0:72) never collide with the staged one-hots (rows 72:105).

Per step: 4 matmuls (2 PSUM column groups x 2 K chunks) + 2 tanh ACTs.
The batch is split into phase-shifted halves so ACT work of one half
overlaps PE work of the other.  Logits matmuls for chunk c-1 are
interleaved into chunk c's steps.
"""

import sys

sys.path.insert(0, "/opt/trn_rl_repo")

import numpy as np
import ml_dtypes

VOCAB = 33
EMBED = 200
HIDDEN = 200
BATCH = 256
SEQ = 1024
NCORES = 8
BPC = BATCH // NCORES          # batch rows per core (32)
H0 = 128
H1 = HIDDEN - H0               # 72
K1A = H1 + VOCAB               # 105: rows of the augmented K1 weight tile

_PROG_CACHE = {}


def _build_program(T, nhalf, tc):
    import concourse.mybir as mybir
    from concourse import bacc, tile
    from concourse.masks import make_identity

    f32 = mybir.dt.float32
    bf16 = mybir.dt.bfloat16
    AF = mybir.ActivationFunctionType
    ALU = mybir.AluOpType

    BH = BPC // nhalf
    assert T % tc == 0
    nchunk = T // tc

    nc = bacc.Bacc(None, target_bir_lowering=False)

    # ---- DRAM I/O ----
    xf_d = [
        nc.dram_tensor(f"xf{h}", [1, (T + 1) * BH], f32, kind="ExternalInput")
        for h in range(nhalf)
    ]
    g0i_d = [
        nc.dram_tensor(f"g0i{h}", [H0, BH], bf16, kind="ExternalInput")
        for h in range(nhalf)
    ]
    g1i_d = [
        nc.dram_tensor(f"g1i{h}", [H1, BH], bf16, kind="ExternalInput")
        for h in range(nhalf)
    ]
    wh0_d = nc.dram_tensor("wh0", [H0, HIDDEN], bf16, kind="ExternalInput")
    wh1_d = nc.dram_tensor("wh1", [H1, HIDDEN], bf16, kind="ExternalInput")
    wo0_d = nc.dram_tensor("wo0", [H0, VOCAB], bf16, kind="ExternalInput")
    wo1_d = nc.dram_tensor("wo1", [H1, VOCAB], bf16, kind="ExternalInput")
    iota_d = nc.dram_tensor("iota", [VOCAB, 1], f32, kind="ExternalInput")
    embT_d = nc.dram_tensor("embT", [EMBED, VOCAB], f32, kind="ExternalInput")
    we_d = nc.dram_tensor("we", [EMBED, HIDDEN], f32, kind="ExternalInput")
    logits_d = nc.dram_tensor("logits", [BPC, T * VOCAB], f32, kind="ExternalOutput")
    hout_d = nc.dram_tensor("hout", [BPC, HIDDEN], f32, kind="ExternalOutput")

    logits_v = logits_d[:].rearrange("b (t v) -> b t v", v=VOCAB)

    with tile.TileContext(nc) as tcx:
        with (
            tcx.tile_pool(name="const", bufs=1) as constp,
            tcx.tile_pool(name="tmp", bufs=1) as tmpp,
            tcx.tile_pool(name="g0p", bufs=2) as g0p,
            tcx.tile_pool(name="ohp", bufs=2) as ohp,
            tcx.tile_pool(name="xbp", bufs=3) as xbp,
            tcx.tile_pool(name="lbp", bufs=4) as lbp,
            tcx.tile_pool(name="ps0p", bufs=3, space="PSUM") as ps0p,
            tcx.tile_pool(name="ps1p", bufs=3, space="PSUM") as ps1p,
            tcx.tile_pool(name="pslgp", bufs=2, space="PSUM") as pslgp,
        ):
            # ---- setup: constants ----
            wh0_t = constp.tile([H0, HIDDEN], bf16, tag="wh0")
            nc.sync.dma_start(wh0_t[:], wh0_d[:])
            whx1_t = constp.tile([K1A, HIDDEN], bf16, tag="whx1")
            nc.sync.dma_start(whx1_t[0:H1, :], wh1_d[:])
            wo0_t = constp.tile([H0, VOCAB], bf16, tag="wo0")
            nc.sync.dma_start(wo0_t[:], wo0_d[:])
            wo1_t = constp.tile([H1, VOCAB], bf16, tag="wo1")
            nc.sync.dma_start(wo1_t[:], wo1_d[:])
            iota_t = constp.tile([VOCAB, 1], f32, tag="iota")
            nc.sync.dma_start(iota_t[:], iota_d[:])
            ident_t = constp.tile([128, 128], bf16, tag="ident")
            make_identity(nc, ident_t[:])

            # EW = embedding @ W_e  (fp32), rounded to bf16 into whx1 rows 72:105
            embT0 = tmpp.tile([H0, VOCAB], f32, tag="embT0")
            nc.sync.dma_start(embT0[:], embT_d[0:H0, :])
            embT1 = tmpp.tile([EMBED - H0, VOCAB], f32, tag="embT1")
            nc.sync.dma_start(embT1[:], embT_d[H0:EMBED, :])
            we0 = tmpp.tile([H0, HIDDEN], f32, tag="we0")
            nc.sync.dma_start(we0[:], we_d[0:H0, :])
            we1 = tmpp.tile([EMBED - H0, HIDDEN], f32, tag="we1")
            nc.sync.dma_start(we1[:], we_d[H0:EMBED, :])
            psew = pslgp.tile([VOCAB, HIDDEN], f32, tag="pslg")
            nc.tensor.matmul(psew[:], embT0[:], we0[:], start=True, stop=False)
            nc.tensor.matmul(psew[:], embT1[:], we1[:], start=False, stop=True)
            ewsb = tmpp.tile([VOCAB, HIDDEN], bf16, tag="ewsb")
            nc.vector.tensor_copy(ewsb[:], psew[:])
            nc.sync.dma_start(whx1_t[H1:K1A, :], ewsb[:])

            # ---- per-half initial state + step-0 one-hot ----
            g0_init = []
            ohprev0 = []
            for h in range(nhalf):
                g0i = constp.tile([H0, BH], bf16, tag=f"g0init{h}")
                nc.sync.dma_start(g0i[:], g0i_d[h][:])
                g0_init.append(g0i)
                ohp0 = constp.tile([K1A, BH], bf16, tag=f"ohprev0{h}")
                nc.sync.dma_start(ohp0[0:H1, :], g1i_d[h][:])
                xb0 = tmpp.tile([VOCAB, BH], f32, tag=f"xb0{h}")
                nc.sync.dma_start(
                    xb0[:], xf_d[h][0:1, 0:BH].broadcast_to([VOCAB, BH])
                )
                oh0s = tmpp.tile([VOCAB, BH], bf16, tag=f"oh0s{h}")
                nc.vector.tensor_scalar(
                    oh0s[:], xb0[:], iota_t[:], None, op0=ALU.is_equal
                )
                nc.sync.dma_start(ohp0[H1:K1A, :], oh0s[:])
                ohprev0.append(ohp0)

            def emit_logits(cprev, hh, bl, g0b, ohb):
                g0v = g0b[:].rearrange("p (t b) -> p t b", b=BH)[:, :, bl]
                ohv = ohb[:].rearrange("p (t b) -> p t b", b=BH)[0:H1, :, bl]
                pl = pslgp.tile([tc, VOCAB], f32, tag="pslg")
                nc.tensor.matmul(pl[:], g0v, wo0_t[:], start=True, stop=False)
                nc.tensor.matmul(pl[:], ohv, wo1_t[:], start=False, stop=True)
                lb = lbp.tile([tc, VOCAB], f32, tag="lb")
                nc.vector.tensor_copy(lb[:], pl[:])
                bglob = hh * BH + bl
                nc.sync.dma_start(
                    logits_v[bglob, cprev * tc:(cprev + 1) * tc, :], lb[:]
                )

            # ---- main loop over chunks ----
            prev = [None] * nhalf  # (g0buf, ohbuf) of previous chunk
            lg_per_step = max(1, (tc // (nhalf * BH)))  # logits tiles per step
            for c in range(nchunk):
                bufs = []
                for h in range(nhalf):
                    g0b = g0p.tile([H0, tc * BH], bf16, tag=f"g0b{h}")
                    ohb = ohp.tile([K1A, tc * BH], bf16, tag=f"ohb{h}")
                    ntok = tc * BH
                    base = (c * tc + 1) * BH
                    for j in range(0, ntok, 512):
                        w = min(512, ntok - j)
                        xbc = xbp.tile([VOCAB, 512], f32, tag=f"xbc{h}")
                        nc.sync.dma_start(
                            xbc[:, 0:w],
                            xf_d[h][0:1, base + j: base + j + w].broadcast_to(
                                [VOCAB, w]
                            ),
                        )
                        ohs = xbp.tile([VOCAB, 512], bf16, tag=f"ohs{h}")
                        nc.vector.tensor_scalar(
                            ohs[:, 0:w], xbc[:, 0:w], iota_t[:], None,
                            op0=ALU.is_equal,
                        )
                        nc.sync.dma_start(ohb[H1:K1A, j:j + w], ohs[:, 0:w])
                    bufs.append((g0b, ohb))

                # how many logits tiles to emit per step for chunk c-1
                n_lg = nhalf * BH  # total b-tiles per chunk
                lg_every = max(1, tc // n_lg)

                for s in range(tc):
                    for h in range(nhalf):
                        g0b, ohb = bufs[h]
                        if c == 0 and s == 0:
                            g0prev = g0_init[h][:]
                            ohprev = ohprev0[h][:]
                        elif s == 0:
                            pg0, poh = prev[h]
                            g0prev = pg0[:, (tc - 1) * BH: tc * BH]
                            ohprev = poh[0:K1A, (tc - 1) * BH: tc * BH]
                        else:
                            g0prev = g0b[:, (s - 1) * BH: s * BH]
                            ohprev = ohb[0:K1A, (s - 1) * BH: s * BH]

                        ps0 = ps0p.tile([H0, BH], f32, tag="ps0")
                        ps1 = ps1p.tile([H1, BH], f32, tag="ps1")
                        # ps1 (rows 128:200 of h) first: its tanh gates the
                        # next step's first matmul.
                        nc.tensor.matmul(
                            ps1[:], whx1_t[:, H0:HIDDEN], ohprev,
                            start=True, stop=False,
                        )
                        nc.tensor.matmul(
                            ps1[:], wh0_t[:, H0:HIDDEN], g0prev,
                            start=False, stop=True,
                        )
                        nc.tensor.matmul(
                            ps0[:], whx1_t[:, 0:H0], ohprev,
                            start=True, stop=False,
                        )
                        nc.tensor.matmul(
                            ps0[:], wh0_t[:, 0:H0], g0prev,
                            start=False, stop=True,
                        )
                        nc.scalar.activation(
                            ohb[0:H1, s * BH:(s + 1) * BH], ps1[:], AF.Tanh
                        )
                        nc.scalar.activation(
                            g0b[:, s * BH:(s + 1) * BH], ps0[:], AF.Tanh
                        )

                    if c >= 1 and s % lg_every == 0:
                        idx = s // lg_every
                        if idx < n_lg:
                            hh = idx // BH
                            bl = idx % BH
                            emit_logits(c - 1, hh, bl, prev[hh][0], prev[hh][1])
                prev = bufs

            # ---- tail: logits for the last chunk ----
            for idx in range(nhalf * BH):
                hh = idx // BH
                bl = idx % BH
                emit_logits(nchunk - 1, hh, bl, prev[hh][0], prev[hh][1])

            # ---- final hidden: transpose g back to [B, H] ----
            for h in range(nhalf):
                pg0, poh = prev[h]
                psh = pslgp.tile([BH, HIDDEN], bf16, tag="pslg")
                nc.tensor.transpose(
                    psh[:, 0:H0], pg0[:, (tc - 1) * BH: tc * BH], ident_t[:]
                )
                nc.tensor.transpose(
                    psh[:, H0:HIDDEN],
                    poh[0:H1, (tc - 1) * BH: tc * BH],
                    ident_t[0:H1, 0:H1],
                )
                hb = tmpp.tile([BH, HIDDEN], f32, tag=f"hb{h}")
                nc.vector.tensor_copy(hb[:], psh[:])
                nc.sync.dma_start(hout_d[h * BH:(h + 1) * BH, :], hb[:])

    nc.compile()
    return nc


def get_program(T=SEQ, nhalf=2, tc=128):
    key = (T, nhalf, tc)
    if key not in _PROG_CACHE:
        _PROG_CACHE[key] = _build_program(T, nhalf, tc)
    return _PROG_CACHE[key]


def _prep_inputs(x, hidden, embedding, W_e, W_h, W_o, T, nhalf):
    """Build the 8 per-core input maps (host-side marshalling only)."""
    bf16 = ml_dtypes.bfloat16
    BH = BPC // nhalf
    x = np.asarray(x)
    hidden = np.asarray(hidden, dtype=np.float32)
    embedding = np.asarray(embedding, dtype=np.float32)
    W_e = np.asarray(W_e, dtype=np.float32)
    W_h = np.asarray(W_h, dtype=np.float32)
    W_o = np.asarray(W_o, dtype=np.float32)

    shared = {
        "wh0": W_h[0:H0, :].astype(bf16),
        "wh1": W_h[H0:HIDDEN, :].astype(bf16),
        "wo0": W_o[0:H0, :].astype(bf16),
        "wo1": W_o[H0:HIDDEN, :].astype(bf16),
        "iota": np.arange(VOCAB, dtype=np.float32).reshape(VOCAB, 1),
        "embT": np.ascontiguousarray(embedding.T),
        "we": W_e,
    }
    in_maps = []
    for core in range(NCORES):
        m = dict(shared)
        xc = x[core * BPC:(core + 1) * BPC, :]        # [32, T]
        hc = hidden[core * BPC:(core + 1) * BPC, :]   # [32, 200]
        for h in range(nhalf):
            xh = xc[h * BH:(h + 1) * BH, :T]          # [BH, T]
            xf = np.concatenate(
                [xh.T.reshape(-1), np.zeros(BH, dtype=xh.dtype)]
            ).astype(np.float32).reshape(1, (T + 1) * BH)
            m[f"xf{h}"] = xf
            gh = np.ascontiguousarray(hc[h * BH:(h + 1) * BH, :].T)  # [200, BH]
            m[f"g0i{h}"] = gh[0:H0, :].astype(bf16)
            m[f"g1i{h}"] = gh[H0:HIDDEN, :].astype(bf16)
        in_maps.append(m)
    return in_maps


def run_on_device(x, hidden, embedding, W_e, W_h, W_o, T=SEQ, nhalf=2, tc=128,
                  trace=False, **kw):
    from concourse.bass_utils import run_bass_kernel_spmd

    tc = min(tc, T)
    nc = get_program(T, nhalf, tc)
    in_maps = _prep_inputs(x, hidden, embedding, W_e, W_h, W_o, T, nhalf)
    res = run_bass_kernel_spmd(
        nc, in_maps, core_ids=list(range(NCORES)), trace=trace, **kw
    )
    logits = np.concatenate(
        [np.asarray(r["logits"], dtype=np.float32).reshape(BPC, T, VOCAB)
         for r in res.results],
        axis=0,
    )
    hout = np.concatenate(
        [np.asarray(r["hout"], dtype=np.float32) for r in res.results], axis=0
    )
    return (logits, hout), res


def kernel(x, hidden, embedding, W_e, W_h, W_o):
    (logits, hout), _ = run_on_device(x, hidden, embedding, W_e, W_h, W_o)
    return logits, hout
